# revision 75
# baseline (speedup 1.0000x reference)
"""GATv2 link-prediction network on 8 TRN2 NeuronCores.

Strategy (edge-parallel, dst-sharded):
  - Nodes padded to 50176 = 8 * 6272; core c owns dst range [c*6272, (c+1)*6272).
  - Edges (incl. self-loops) sorted by dst, assigned to the core owning dst,
    grouped into 49 dst-windows of 128 nodes, each padded to SB*128 edge slots.
  - Per layer: per-node tables xl = x@wl, xr = x@wr computed locally and
    AllGathered; per edge-subtile the src rows are fetched with streamed
    indirect DMAs; dst rows are expanded on-chip from the 128-row dst window
    with a selection-matrix matmul.
  - Attention logits: e = a . leaky_relu(u+v) via wide DVE ops; w = exp(e)
    (softmax max-subtraction dropped: |e| <= ~10 so fp32 exp is exact enough).
  - Segment softmax + aggregation fused into PSUM matmuls:
    psum[d, :] += (S_T * w).T @ [u | 1]  ->  z[d] = psum[:, :F]/psum[:, F] + b.
  - Decoder: z2 rows gathered per decode edge, MLP runs feature-major on PE.

Call-layer performance (the dominant cost: device exec is ~20ms, while the
loopback PJRT relay moves ~140MB/s with ~100ms blocking latency, all sharing
one host CPU): per-call bytes are minimized — x as fp8 e3m4 (exact-cast to
bf16 on-device), edge src ids as u16 + dst-local ids as u8 in one packed
byte tensor (bitcast + widened on-device), decode ids as u16, weights
uploaded once as a 16-row shard and AllGathered on-device, f32 constants in
one row (PE outer-product broadcast / transpose on-device), output as f16.
Host prep is fused into numba single-pass kernels and each tensor is
device_put asynchronously the moment it is assembled so upload streams
behind the remaining CPU work. The executor is built once and reused:
run_bass_kernel_spmd's axon path (bass2jax.run_bass_via_pjrt) re-creates the
jax.jit wrapper on every call, which costs seconds of re-trace/re-lower; we
inline that same path with a persistent jit. The donated output zero-buffers
are recycled from the previous call's device output (the kernel writes every
output element, so stale contents are harmless).
"""

import sys

sys.path.insert(0, "/opt/trn_rl_repo")

import numpy as np
import ml_dtypes

import concourse.bacc as bacc
import concourse.bass as bass
import concourse.mybir as mybir
import concourse.tile as tile

BF16 = mybir.dt.bfloat16
F32 = mybir.dt.float32
F16 = mybir.dt.float16
F8E3 = mybir.dt.float8e3
I32 = mybir.dt.int32
U16 = mybir.dt.uint16
U8 = mybir.dt.uint8

NC = 8
NEG_SLOPE = 0.2


class Cfg:
    def __init__(self, n=50000, e=1600000, e_dec=500000, in_c=128, hid=128,
                 out_c=64, sb=36, dec_t=512):
        self.N, self.E, self.E_DEC = n, e, e_dec
        self.IN_C, self.HID, self.OUT_C = in_c, hid, out_c
        self.NPC = ((n // NC + 127) // 128) * 128      # padded nodes per core
        self.G = self.NPC // 128                        # dst groups per core
        self.NP = self.NPC * NC                         # padded node count
        self.SB = sb                                    # subtiles per group
        self.W = sb * 128                               # edge slots per group
        self.DEC_T = dec_t                              # decode edges per tile
        dec_pc = (2 * e_dec) // NC
        self.DEC_PC = dec_pc
        self.DEC_NT = (dec_pc + dec_t - 1) // dec_t     # decode tiles per core
        self.DEC_PAD = self.DEC_NT * dec_t


CFG_FULL = Cfg()

try:
    import numba

    @numba.njit(cache=True)
    def _edge_pack(e0, e1, n_nodes, ED, DLR, npcr, NPC, SB, ngg):
        """Count-then-scatter edge grouping (incl. self-loops), one core pass.

        ED is the flat uint8 view of [ngg, 128, 2*SB]: little-endian u16 src
        offsets. DLR is the flat view of [ngg, SB*128]: j-major dst-local
        ids (255 = padding, pre-filled).
        """
        ne = e0.shape[0]
        cursor = np.zeros(ngg, np.int32)
        row = 2 * SB
        for e in range(ne):
            s = e0[e]
            d = e1[e]
            qs = s // npcr
            sp = qs * NPC + (s - qs * npcr)
            qd = d // npcr
            dp = qd * NPC + (d - qd * npcr)
            g = dp >> 7
            slot = cursor[g]
            cursor[g] = slot + 1
            base = (g * 128 + (slot & 127)) * row
            j = slot >> 7
            ED[base + 2 * j] = sp & 255
            ED[base + 2 * j + 1] = sp >> 8
            DLR[(g * SB + j) * 128 + (slot & 127)] = dp & 127
        for i in range(n_nodes):
            q = i // npcr
            dp = q * NPC + (i - q * npcr)
            g = dp >> 7
            slot = cursor[g]
            cursor[g] = slot + 1
            base = (g * 128 + (slot & 127)) * row
            j = slot >> 7
            ED[base + 2 * j] = dp & 255
            ED[base + 2 * j + 1] = dp >> 8
            DLR[(g * SB + j) * 128 + (slot & 127)] = dp & 127
        mx = 0
        for g in range(ngg):
            if cursor[g] > mx:
                mx = cursor[g]
        return mx
    @numba.njit(cache=True)
    def _x_to_f8_nb(xu, XL, lut, n_nodes, npcr, NPC, in_c):
        """f32 bits -> round-to-nearest bf16 index -> f8e3m4 byte, written
        into the node-padded per-core layout in one pass."""
        for n_ in range(n_nodes):
            cc = n_ // npcr
            ob = (cc * NPC + (n_ - cc * npcr)) * in_c
            ib = n_ * in_c
            for f in range(in_c):
                XL[ob + f] = lut[(xu[ib + f] + 0x8000) >> 16]

    @numba.njit(cache=True)
    def _dec_pack2_nb(pa, pb, na, nb, out_, npcr, NPC, DEC_PC, DEC_T,
                      DEC_NT, DGC):
        """Both sides of pos then neg decode edges in two passes; writing
        side a and b of the same edge back-to-back hits the same cache line
        of out_ ([tile, p, side, k] layout)."""
        ed = pa.shape[0]
        for half in range(2):
            ia = pa if half == 0 else na
            ib = pb if half == 0 else nb
            off = half * ed
            for i in range(ed):
                gi = off + i
                c = gi // DEC_PC
                ric = gi - c * DEC_PC
                t = ric // DEC_T
                r = ric - t * DEC_T
                base = ((c * DEC_NT + t) * 128 + (r & 127)) * 2 * DGC + (r >> 7)
                v = ia[i]
                q = v // npcr
                out_[base] = q * NPC + (v - q * npcr)
                v = ib[i]
                q = v // npcr
                out_[base + DGC] = q * NPC + (v - q * npcr)
except ImportError:  # pragma: no cover - numba always present in container
    _edge_pack = None
    _dec_pack2_nb = None
    _x_to_f8_nb = None


_F8LUT = None


def _f8lut():
    """u8 f8e3m4 bytes indexed by bf16 bit pattern."""
    global _F8LUT
    if _F8LUT is None:
        allbf = (np.arange(65536, dtype=np.uint32) << 16).view(np.float32)
        _F8LUT = np.ascontiguousarray(
            allbf.astype(ml_dtypes.float8_e3m4).view(np.uint8))
    return _F8LUT


def build_kernel(c: Cfg):
    nc = bacc.Bacc("TRN2", num_devices=NC)
    SB, G, NPC, NP = c.SB, c.G, c.NPC, c.NP
    IN_C, HID, OUT_C = c.IN_C, c.HID, c.OUT_C
    DEC_T, DEC_NT = c.DEC_T, c.DEC_NT
    DGC = DEC_T // 128                                  # gather calls per side per tile

    # ---- I/O (5 inputs: per-put issue overhead over the tunnel is ~15ms,
    # so everything small is packed into wpack/cvec) ----
    x_loc = nc.dram_tensor("x_loc", [NPC, IN_C], F8E3, kind="ExternalInput")
    # per group: 2*SB bytes of u16 src offsets per partition row
    edges = nc.dram_tensor("edges", [G, 128, 2 * SB], U8, kind="ExternalInput")
    # dst-local ids, j-major flat row per group (plain 2-D DMA loads only:
    # a 1-D rearranged-AP DMA makes the NEFF unloadable)
    dlrow = nc.dram_tensor("dlrow", [G, SB * 128], U8, kind="ExternalInput")
    # per decode tile: DGC cols of side-a offsets | DGC cols of side-b
    offs_ab = nc.dram_tensor("offs_ab", [DEC_NT, 128, 2 * DGC], U16,
                             kind="ExternalInput")
    # w1lr | w2lr | fw1 | fw2(pad) | fw3 | fw4(pad) — each core gets a
    # 16-row slice; the full [128, 704] is AllGathered on-device
    wpack = nc.dram_tensor("wpack", [128 // NC, 704], BF16,
                           kind="ExternalInput")
    wscr = nc.dram_tensor("wscr", [128 // NC, 704], BF16)
    wfull = nc.dram_tensor("wfull", [128, 704], BF16, addr_space="Shared")
    # a1|b1|a2|b2 | fb1(pad) | fb2 | fb3(pad) | fb4(pad)
    cvec = nc.dram_tensor("cvec", [1, 1024], F32, kind="ExternalInput")
    out = nc.dram_tensor("out", [DEC_NT, DEC_T], F16, kind="ExternalOutput")

    # internal DRAM
    xl1_loc = nc.dram_tensor("xl1_loc", [NPC, HID], BF16)
    xr1_loc = nc.dram_tensor("xr1_loc", [NPC, HID], BF16)
    xl1 = nc.dram_tensor("xl1", [NP, HID], BF16, addr_space="Shared")
    z1_loc = nc.dram_tensor("z1_loc", [NPC, HID], BF16)
    xl2_loc = nc.dram_tensor("xl2_loc", [NPC, OUT_C], BF16)
    xl2 = nc.dram_tensor("xl2", [NP, OUT_C], BF16, addr_space="Shared")
    xr2_loc = nc.dram_tensor("xr2_loc", [NPC, OUT_C], BF16)
    z2_loc = nc.dram_tensor("z2_loc", [NPC, OUT_C], BF16)
    z2 = nc.dram_tensor("z2", [NP, OUT_C], BF16, addr_space="Shared")

    rg = [list(range(NC))]

    with tile.TileContext(nc) as tc:
        with tc.tile_pool(name="const", bufs=1) as cp, \
             tc.tile_pool(name="sb", bufs=2) as sp, \
             tc.tile_pool(name="wide", bufs=2) as wp, \
             tc.tile_pool(name="wide1", bufs=1) as wq, \
             tc.tile_pool(name="ps", bufs=2, space="PSUM") as pp, \
             tc.tile_pool(name="ps2", bufs=2, space="PSUM") as pp2, \
             tc.tile_pool(name="ps3", bufs=3, space="PSUM") as pp3:

            ident = cp.tile([128, 128], BF16, tag="ident")
            from concourse.masks import make_identity
            make_identity(nc, ident[:])
            # iota row 0..127, same on every partition, generated on-device
            iota_i = cp.tile([128, 128], I32, tag="iota_i")
            nc.gpsimd.iota(iota_i[:], pattern=[[1, 128]], base=0,
                           channel_multiplier=0)
            iota_t = cp.tile([128, 128], BF16, tag="iota")
            nc.vector.tensor_copy(out=iota_t[:], in_=iota_i[:])
            iota_wi = sp.tile([128, SB * 128], I32, tag="iota_wi")
            nc.gpsimd.iota(iota_wi[:], pattern=[[0, SB * 128]], base=0,
                           channel_multiplier=1)
            iota_w = cp.tile([128, SB * 128], BF16, tag="iota_w")
            nc.vector.tensor_copy(out=iota_w[:], in_=iota_wi[:])
            # one upload slice per core -> AllGather -> one DMA for all weights
            # (collectives can't read IO tensors: stage via SBUF -> scratch)
            wsl = cp.tile([128 // NC, 704], BF16, tag="wslice")
            nc.sync.dma_start(out=wsl[:], in_=wpack[:])
            nc.sync.dma_start(out=wscr[:], in_=wsl[:])
            nc.gpsimd.collective_compute(
                "AllGather", mybir.AluOpType.bypass,
                replica_groups=[list(range(NC))],
                ins=[wscr[:]], outs=[wfull[:]])
            wl_t = cp.tile([128, 704], BF16, tag="wpack")
            nc.sync.dma_start(out=wl_t[:], in_=wfull[:])
            w1_t = wl_t[:, 0:256]
            w2_t = wl_t[:, 256:384]
            fw1_t = wl_t[:, 384:448]
            fw2_t = wl_t[:64, 448:576]
            fw3_t = wl_t[:, 576:640]
            fw4_t = wl_t[:64, 640:704]
            cv_t = cp.tile([1, 1024], F32, tag="cvec")
            nc.sync.dma_start(out=cv_t[:], in_=cvec[:])
            # broadcast a1/b1/a2/b2 rows [1,128] -> [128,128] via PE outer product
            ones1 = cp.tile([1, 128], F32, tag="ones1")
            nc.vector.memset(ones1[:], 1.0)
            ab_bc = []
            for i in range(4):
                psb = pp3.tile([128, 128], F32, tag="C")
                nc.tensor.matmul(out=psb[:], lhsT=ones1[:],
                                 rhs=cv_t[0:1, i * 128:(i + 1) * 128],
                                 start=True, stop=True)
                tbc = cp.tile([128, 128], F32, tag=f"abbc{i}")
                nc.vector.tensor_copy(out=tbc[:], in_=psb[:])
                ab_bc.append(tbc)
            a1_t, b1_t, a2_t, b2_t = ab_bc
            # decoder bias columns: PE-transpose each [1,128] row to [128,1]
            ones11 = cp.tile([1, 1], F32, tag="ones11")
            nc.vector.memset(ones11[:], 1.0)
            fb_t = cp.tile([128, 4], F32, tag="fb")
            for i in range(4):
                psc = pp3.tile([128, 128], F32, tag="C")
                nc.tensor.transpose(out=psc[:, 0:1],
                                    in_=cv_t[0:1, 512 + i * 128:
                                             512 + (i + 1) * 128],
                                    identity=ones11[:])
                nc.vector.tensor_copy(out=fb_t[:, i:i + 1], in_=psc[:, 0:1])

            def tables(src_dram, w_t, fin, fout2, dst_l, dst_r, in_dt=BF16):
                """dst_l[i] | dst_r[i] = (src[i*128:...]) @ [wl | wr]."""
                ntile = src_dram.shape[0] // 128
                for i in range(ntile):
                    if in_dt is BF16:
                        xt = sp.tile([128, fin], BF16, tag="tab_x")
                        nc.sync.dma_start(out=xt[:],
                                          in_=src_dram[i * 128:(i + 1) * 128, :])
                    else:
                        x8 = sp.tile([128, fin], in_dt, tag="tab_x8")
                        nc.sync.dma_start(out=x8[:],
                                          in_=src_dram[i * 128:(i + 1) * 128, :])
                        xt = sp.tile([128, fin], BF16, tag="tab_x")
                        nc.vector.tensor_copy(out=xt[:], in_=x8[:])
                    xtt = pp.tile([fin, 128], BF16, tag="A")
                    nc.tensor.transpose(out=xtt[:], in_=xt[:], identity=ident[:])
                    xts = sp.tile([fin, 128], BF16, tag="tab_Ts")
                    nc.vector.tensor_copy(out=xts[:], in_=xtt[:])
                    op = pp2.tile([128, fout2], F32, tag="B")
                    nc.tensor.matmul(out=op[:], lhsT=xts[:], rhs=w_t[:],
                                     start=True, stop=True)
                    os_ = sp.tile([128, fout2], BF16, tag="tab_os")
                    nc.vector.tensor_copy(out=os_[:], in_=op[:])
                    nc.sync.dma_start(out=dst_l[i * 128:(i + 1) * 128, :],
                                      in_=os_[:, :fout2 // 2])
                    nc.sync.dma_start(out=dst_r[i * 128:(i + 1) * 128, :],
                                      in_=os_[:, fout2 // 2:])

            def allgather(loc, full):
                nc.gpsimd.collective_compute(
                    "AllGather", mybir.AluOpType.bypass, replica_groups=rg,
                    ins=[loc[:]], outs=[full[:]])

            def edge_layer(ul_tab, vloc_tab, F_, a_t, b_t, relu, z_out):
                """One GATv2 layer edge pass. F_ = feature width."""
                FE = F_ + 4                      # u tile row: F_ feats + 1.0 col + pad
                for g in range(G):
                    ed = sp.tile([128, 2 * SB], U8, tag="edges8")
                    nc.gpsimd.dma_start(out=ed[:], in_=edges[g])
                    ou = sp.tile([128, SB], I32, tag="offu")
                    nc.vector.tensor_copy(out=ou[:], in_=ed[:, :].bitcast(U16))
                    dlf8 = sp.tile([1, SB * 128], U8, tag="dlf8")
                    nc.sync.dma_start(out=dlf8[:], in_=dlrow[g:g + 1, :])
                    dlf = sp.tile([1, SB * 128], BF16, tag="dlf")
                    nc.vector.tensor_copy(out=dlf[:], in_=dlf8[:])
                    dlbc = wq.tile([128, SB * 128], BF16, tag="dlbc")
                    nc.gpsimd.partition_broadcast(dlbc[:], dlf[:])
                    dlj8 = sp.tile([SB, 128], U8, tag="dlj8")
                    nc.sync.dma_start(
                        out=dlj8[:],
                        in_=dlrow[g:g + 1, :].rearrange("o (j s) -> (o j) s",
                                                        j=SB))
                    dljb = sp.tile([SB, 128], BF16, tag="dljb")
                    nc.vector.tensor_copy(out=dljb[:], in_=dlj8[:])
                    dlp = pp3.tile([128, 128], BF16, tag="C")
                    nc.tensor.transpose(out=dlp[:, :SB], in_=dljb[:],
                                        identity=ident[:SB, :SB])
                    dl = sp.tile([128, SB], BF16, tag="dstloc")
                    nc.vector.tensor_copy(out=dl[:], in_=dlp[:, :SB])
                    u = wp.tile([128, SB * FE], BF16, tag="u")
                    u3 = u[:].rearrange("p (j f) -> p j f", j=SB)
                    nc.vector.memset(u3[:, :, F_:F_ + 1], 1.0)
                    for j in range(SB):
                        nc.gpsimd.indirect_dma_start(
                            out=u3[:, j, :F_], out_offset=None, in_=ul_tab[:],
                            in_offset=bass.IndirectOffsetOnAxis(
                                ap=ou[:, j:j + 1], axis=0))
                    t = wp.tile([128, SB * F_], F32, tag="t")
                    t3 = t[:].rearrange("p (j f) -> p j f", j=SB)
                    st = wp.tile([128, SB * 128], BF16, tag="st")
                    st3 = st[:].rearrange("p (j d) -> p j d", j=SB)
                    nc.vector.tensor_tensor(
                        out=st3[:, :, :],
                        in0=dl[:].rearrange("p (j o) -> p j o", o=1).to_broadcast([128, SB, 128]),
                        in1=iota_t[:].rearrange("p (o d) -> p o d", o=1).to_broadcast([128, SB, 128]),
                        op=mybir.AluOpType.is_equal)
                    # v rows for this dst window, expanded per-edge on PE.
                    # The selection matrix is built directly in transposed,
                    # j-major form (one wide compare vs 36 transposes+copies)
                    vg = sp.tile([128, F_], BF16, tag="vg")
                    nc.sync.dma_start(
                        out=vg[:], in_=vloc_tab[g * 128:(g + 1) * 128, :])
                    stT = wq.tile([128, SB * 128], BF16, tag="stT")
                    stT3 = stT[:].rearrange("p (j s) -> p j s", j=SB)
                    nc.vector.tensor_tensor(out=stT[:], in0=dlbc[:],
                                            in1=iota_w[:],
                                            op=mybir.AluOpType.is_equal)
                    for j in range(SB):
                        vp = pp2.tile([128, F_], F32, tag="B")
                        nc.tensor.matmul(out=vp[:], lhsT=stT3[:, j, :],
                                         rhs=vg[:], start=True, stop=True)
                        nc.vector.tensor_add(out=t3[:, j, :],
                                             in0=u3[:, j, :F_], in1=vp[:])
                    nc.vector.scalar_tensor_tensor(
                        out=t[:], in0=t[:], scalar=float(NEG_SLOPE), in1=t[:],
                        op0=mybir.AluOpType.mult, op1=mybir.AluOpType.max)
                    # t is dead after the a-weighting: multiply in place
                    ta = t
                    nc.vector.tensor_tensor(
                        out=ta[:].rearrange("p (j f) -> p j f", j=SB),
                        in0=t3[:, :, :],
                        in1=a_t[:, :F_].rearrange("p (o f) -> p o f", o=1).to_broadcast([128, SB, F_]),
                        op=mybir.AluOpType.mult)
                    ev = sp.tile([128, SB], F32, tag="ev")
                    nc.vector.tensor_reduce(
                        out=ev[:], in_=ta[:].rearrange("p (j f) -> p j f", j=SB),
                        axis=mybir.AxisListType.X, op=mybir.AluOpType.add)
                    wv = sp.tile([128, SB], F32, tag="wv")
                    nc.scalar.activation(wv[:], ev[:],
                                         mybir.ActivationFunctionType.Exp)
                    # S' = S_T * w  (broadcast w along d)
                    nc.vector.tensor_tensor(
                        out=st3[:, :, :], in0=st3[:, :, :],
                        in1=wv[:].rearrange("p (j o) -> p j o", o=1).to_broadcast([128, SB, 128]),
                        op=mybir.AluOpType.mult)
                    acc = pp.tile([128, F_ + 4], F32, tag="A")
                    for j in range(SB):
                        nc.tensor.matmul(
                            out=acc[:, :F_ + 1], lhsT=st3[:, j, :],
                            rhs=u3[:, j, :F_ + 1],
                            start=(j == 0), stop=(j == SB - 1))
                    den = sp.tile([128, 1], F32, tag="den")
                    nc.vector.tensor_scalar_add(den[:], acc[:, F_:F_ + 1], 1e-30)
                    rec = sp.tile([128, 1], F32, tag="rec")
                    nc.vector.reciprocal(rec[:], den[:])
                    zt = sp.tile([128, F_], F32, tag="zt")
                    nc.vector.scalar_tensor_tensor(
                        out=zt[:], in0=acc[:, :F_], scalar=rec[:, :1], in1=b_t[:, :F_],
                        op0=mybir.AluOpType.mult, op1=mybir.AluOpType.add)
                    zb = sp.tile([128, F_], BF16, tag="zb")
                    if relu:
                        nc.scalar.activation(zb[:], zt[:],
                                             mybir.ActivationFunctionType.Relu)
                    else:
                        nc.vector.tensor_copy(out=zb[:], in_=zt[:])
                    nc.sync.dma_start(out=z_out[g * 128:(g + 1) * 128, :], in_=zb[:])

            # ---- phase A: L1 tables (xr1 only needed locally) ----
            tables(x_loc, w1_t, IN_C, 2 * HID, xl1_loc, xr1_loc, in_dt=F8E3)
            allgather(xl1_loc, xl1)
            # ---- phase B: L1 edges ----
            edge_layer(xl1, xr1_loc, HID, a1_t, b1_t, True, z1_loc)
            # ---- phase D: L2 tables, computed on local z1 rows only and
            # AllGathered (vs recomputing the full table on every core) ----
            tables(z1_loc, w2_t, HID, 2 * OUT_C, xl2_loc, xr2_loc)
            allgather(xl2_loc, xl2)
            # ---- phase E: L2 edges ----
            edge_layer(xl2, xr2_loc, OUT_C, a2_t, b2_t, False, z2_loc)
            allgather(z2_loc, z2)

            # ---- decoder ----
            for tdx in range(DEC_NT):
                ab16 = sp.tile([128, 2 * DGC], U16, tag="offab16")
                nc.gpsimd.dma_start(out=ab16[:], in_=offs_ab[tdx])
                oa = sp.tile([128, DGC], I32, tag="offa")
                nc.vector.tensor_copy(out=oa[:], in_=ab16[:, :DGC])
                ob = sp.tile([128, DGC], I32, tag="offb")
                nc.vector.tensor_copy(out=ob[:], in_=ab16[:, DGC:])
                h = wp.tile([128, DGC * 2 * OUT_C], BF16, tag="h")
                h3 = h[:].rearrange("p (k f) -> p k f", k=DGC)
                for k in range(DGC):
                    nc.gpsimd.indirect_dma_start(
                        out=h3[:, k, :OUT_C], out_offset=None, in_=z2[:],
                        in_offset=bass.IndirectOffsetOnAxis(ap=oa[:, k:k + 1], axis=0))
                    nc.gpsimd.indirect_dma_start(
                        out=h3[:, k, OUT_C:], out_offset=None, in_=z2[:],
                        in_offset=bass.IndirectOffsetOnAxis(ap=ob[:, k:k + 1], axis=0))
                hT = sp.tile([128, DEC_T], BF16, tag="hT")
                for k in range(DGC):
                    htp = pp3.tile([128, 128], BF16, tag="C")
                    nc.tensor.transpose(out=htp[:], in_=h3[:, k, :], identity=ident[:])
                    nc.vector.tensor_copy(out=hT[:, k * 128:(k + 1) * 128], in_=htp[:])
                p1 = pp.tile([OUT_C, DEC_T], F32, tag="A")
                nc.tensor.matmul(out=p1[:], lhsT=fw1_t[:], rhs=hT[:], start=True, stop=True)
                s1 = sp.tile([OUT_C, DEC_T], BF16, tag="mlps1")
                nc.scalar.activation(s1[:], p1[:], mybir.ActivationFunctionType.Relu,
                                     bias=fb_t[:OUT_C, 0:1])
                p2 = pp2.tile([128, DEC_T], F32, tag="B")
                nc.tensor.matmul(out=p2[:], lhsT=fw2_t[:], rhs=s1[:], start=True, stop=True)
                s2 = sp.tile([128, DEC_T], BF16, tag="mlps2")
                nc.scalar.activation(s2[:], p2[:], mybir.ActivationFunctionType.Relu,
                                     bias=fb_t[:128, 1:2])
                p3 = pp3.tile([64, DEC_T], F32, tag="C")
                nc.tensor.matmul(out=p3[:], lhsT=fw3_t[:], rhs=s2[:], start=True, stop=True)
                s3 = sp.tile([64, DEC_T], BF16, tag="mlps3")
                nc.scalar.activation(s3[:], p3[:], mybir.ActivationFunctionType.Relu,
                                     bias=fb_t[:64, 2:3])
                p4 = pp.tile([64, DEC_T], F32, tag="A")
                nc.tensor.matmul(out=p4[:], lhsT=fw4_t[:], rhs=s3[:], start=True, stop=True)
                s4 = sp.tile([1, DEC_T], F16, tag="s4")
                nc.vector.tensor_scalar_add(s4[:], p4[:1, :], fb_t[:1, 3:4])
                nc.sync.dma_start(out=out[tdx:tdx + 1, :], in_=s4[:])

    nc.compile()
    return nc


# ---------------- host side ----------------

def _prep(c: Cfg, inputs, put=lambda a: a):
    """Shard + pad inputs; returns {name: array}.

    `put` is applied to each finished tensor immediately, so an async
    jax.device_put can stream earlier tensors while later ones are still
    being assembled on the CPU (x_loc is 37% of the bytes and is ready
    first; the edge grouping below then overlaps its upload).
    """
    bf = ml_dtypes.bfloat16
    N, NPC, G, SB, NP = c.N, c.NPC, c.G, c.SB, c.NP
    npc_real = N // NC
    named = {}

    def pid(n):
        q, r = np.divmod(n.astype(np.int32, copy=False), npc_real)
        return q * NPC + r

    # ---- replicated weights (near-instant: gets the tunnel streaming) ----
    def rep(a):
        return np.ascontiguousarray(np.broadcast_to(a, (NC,) + a.shape)).reshape(
            (NC * a.shape[0],) + a.shape[1:])

    wp_ = np.zeros((128, 704), np.float32)
    wp_[:, 0:128] = np.asarray(inputs["w1l"], np.float32)
    wp_[:, 128:256] = np.asarray(inputs["w1r"], np.float32)
    wp_[:, 256:320] = np.asarray(inputs["w2l"], np.float32)
    wp_[:, 320:384] = np.asarray(inputs["w2r"], np.float32)
    wp_[:, 384:448] = np.asarray(inputs["fw1"], np.float32)
    wp_[:64, 448:576] = np.asarray(inputs["fw2"], np.float32)
    wp_[:, 576:640] = np.asarray(inputs["fw3"], np.float32)
    wp_[:64, 640:641] = np.asarray(inputs["fw4"], np.float32)
    named["wpack"] = put(wp_.astype(bf))  # sharded 16 rows/core, allgathered
    cv = np.zeros((1, 1024), np.float32)
    cv[0, 0:c.HID] = np.asarray(inputs["a1"], np.float32)
    cv[0, 128:128 + c.HID] = np.asarray(inputs["b1"], np.float32)
    cv[0, 256:256 + c.OUT_C] = np.asarray(inputs["a2"], np.float32)
    cv[0, 384:384 + c.OUT_C] = np.asarray(inputs["b2"], np.float32)
    cv[0, 512:512 + c.OUT_C] = np.asarray(inputs["fb1"], np.float32)
    cv[0, 640:768] = np.asarray(inputs["fb2"], np.float32)
    cv[0, 768:768 + 64] = np.asarray(inputs["fb3"], np.float32)
    cv[0, 896:897] = np.asarray(inputs["fb4"], np.float32)
    named["cvec"] = put(rep(cv))

    # ---- nodes (cheap to build, big to ship) ----
    x = np.ascontiguousarray(np.asarray(inputs["x"], np.float32))
    lut = _f8lut()
    XL = np.zeros(NC * NPC * c.IN_C, np.uint8)
    if _x_to_f8_nb is not None:
        _x_to_f8_nb(x.reshape(-1).view(np.uint32), XL, lut, N, npc_real,
                    NPC, c.IN_C)
    else:
        idx = (x.reshape(-1).view(np.uint32) + 0x8000) >> 16
        XLv = XL.reshape(NC, NPC, c.IN_C)
        XLv[:, :npc_real] = lut[idx].reshape(NC, npc_real, c.IN_C)
    named["x_loc"] = put(
        XL.view(ml_dtypes.float8_e3m4).reshape(NC * NPC, c.IN_C))

    # ---- decode edges, data-parallel ----
    pe = np.asarray(inputs["pos_edge_index"])
    ne = np.asarray(inputs["neg_edge_index"])
    DGC = c.DEC_T // 128
    OAB = np.zeros(NC * c.DEC_NT * 128 * 2 * DGC, np.uint16)
    if _dec_pack2_nb is not None:
        _dec_pack2_nb(np.ascontiguousarray(pe[0].astype(np.int32, copy=False)),
                      np.ascontiguousarray(pe[1].astype(np.int32, copy=False)),
                      np.ascontiguousarray(ne[0].astype(np.int32, copy=False)),
                      np.ascontiguousarray(ne[1].astype(np.int32, copy=False)),
                      OAB, npc_real, NPC, c.DEC_PC, c.DEC_T, c.DEC_NT, DGC)
    else:
        v = OAB.reshape(NC * c.DEC_NT, 2, DGC, 128)  # [tile, side, k, p]
        for side, (p_, n_) in enumerate([(pe[0], ne[0]), (pe[1], ne[1])]):
            ids = np.concatenate([p_, n_]).astype(np.int32, copy=False)
            arr = np.zeros((NC, c.DEC_PAD), np.uint16)
            arr[:, :c.DEC_PC] = pid(ids).reshape(NC, c.DEC_PC)
            v[:, side] = arr.reshape(NC * c.DEC_NT, DGC, 128)
        OAB = np.ascontiguousarray(
            v.transpose(0, 3, 1, 2)).reshape(-1)  # -> [tile, p, side, k]
    named["offs_ab"] = put(OAB.reshape(NC * c.DEC_NT, 128, 2 * DGC))

    # ---- message edges, grouped by 128-wide dst window ----
    ei = np.asarray(inputs["edge_index"])
    e0 = np.ascontiguousarray(ei[0].astype(np.int32, copy=False))
    e1 = np.ascontiguousarray(ei[1].astype(np.int32, copy=False))
    ngg = NC * G
    ED = np.zeros(ngg * 128 * 2 * SB, np.uint8)
    DLR = np.full(ngg * SB * 128, 255, np.uint8)
    if _edge_pack is not None:
        mx = _edge_pack(e0, e1, N, ED, DLR, npc_real, NPC, SB, ngg)
        assert mx <= SB * 128, f"group overflow: {mx} > {SB * 128}"
    else:
        loops = np.arange(N, dtype=np.int32)
        src = np.concatenate([e0, loops])
        dst = np.concatenate([e1, loops])
        sp_, dp = pid(src), pid(dst)
        gg_e = (dp >> 7).astype(np.uint16)   # NPC % 128 == 0 -> global group id
        order = np.argsort(gg_e, kind="stable")
        sp_s = sp_[order].astype(np.uint16)
        dl_s = (dp[order] & 127).astype(np.uint8)
        counts = np.bincount(gg_e, minlength=ngg)
        assert counts.max() <= SB * 128, \
            f"group overflow: {counts.max()} > {SB * 128}"
        starts = np.zeros(ngg, np.int32)
        np.cumsum(counts[:-1], out=starts[1:], dtype=np.int32)
        slot = np.arange(dp.shape[0], dtype=np.int32) - np.repeat(starts, counts)
        gg = gg_e[order].astype(np.int32)
        rowbase = (gg * 128 + slot % 128) * (2 * SB)
        j_ = slot // 128
        ED[rowbase + 2 * j_] = (sp_s & 255).astype(np.uint8)
        ED[rowbase + 2 * j_ + 1] = (sp_s >> 8).astype(np.uint8)
        DLR[(gg * SB + j_) * 128 + slot % 128] = dl_s
    named["edges"] = put(ED.reshape(ngg, 128, 2 * SB))
    named["dlrow"] = put(DLR.reshape(ngg, SB * 128))
    return named


class _Exec:
    """Persistent jit wrapper around the bass NEFF (the same PJRT path
    run_bass_kernel_spmd takes under axon, minus the per-call re-trace)."""

    def __init__(self, nc):
        import jax
        from jax.sharding import Mesh, PartitionSpec
        from jax.experimental.shard_map import shard_map
        from concourse import bass2jax

        bass2jax.install_neuronx_cc_hook()
        self.jax = jax
        partition_name = (nc.partition_id_tensor.name
                          if nc.partition_id_tensor else None)
        in_names, out_names, out_avals, zero_outs = [], [], [], []
        for alloc in nc.m.functions[0].allocations:
            if not isinstance(alloc, mybir.MemoryLocationSet):
                continue
            name = alloc.memorylocations[0].name
            if alloc.kind == "ExternalInput":
                if name != partition_name:
                    in_names.append(name)
            elif alloc.kind == "ExternalOutput":
                shape = tuple(alloc.tensor_shape)
                dtype = mybir.dt.np(alloc.dtype)
                out_names.append(name)
                out_avals.append(jax.core.ShapedArray(shape, dtype))
                zero_outs.append(
                    np.zeros((NC * shape[0], *shape[1:]), dtype))
        n_params = len(in_names)
        self.in_names = list(in_names)
        self.out_names = out_names
        all_names = in_names + out_names
        if partition_name is not None:
            all_names.append(partition_name)
        donate = tuple(range(n_params, n_params + len(out_names)))

        def _body(*args):
            operands = list(args)
            if partition_name is not None:
                operands.append(bass2jax.partition_id_tensor())
            return tuple(_bind(*operands))

        def _bind(*operands):
            return bass2jax._bass_exec_p.bind(
                *operands, out_avals=tuple(out_avals),
                in_names=tuple(all_names), out_names=tuple(out_names),
                lowering_input_output_aliases=(), sim_require_finite=True,
                sim_require_nnan=True, nc=nc)

        devices = jax.devices()[:NC]
        mesh = Mesh(np.asarray(devices), ("core",))
        specs = (PartitionSpec("core"),)
        self.sharded = jax.jit(
            shard_map(_body, mesh=mesh,
                      in_specs=specs * (n_params + len(out_names)),
                      out_specs=specs * len(out_names), check_rep=False),
            donate_argnums=donate, keep_unused=True)
        # pre-place the first call's donated out-buffers so every call sees
        # device-array outbufs (one jit signature, no second XLA compile)
        from jax.sharding import NamedSharding
        self.shd = NamedSharding(mesh, PartitionSpec("core"))
        self._next_outbufs = [jax.device_put(z, self.shd) for z in zero_outs]

    def put(self, arr):
        return self.jax.device_put(arr, self.shd)

    def __call__(self, named):
        import time
        args = [named[n] for n in self.in_names]
        outs = self.sharded(*args, *self._next_outbufs)
        # prefetch D2H, then wait by yielding: the blocking asarray path
        # busy-holds the only CPU that the loopback relay needs to finish
        # streaming; a sleep-spin is ~10-15ms faster end-to-end
        try:
            for o in outs:
                o.copy_to_host_async()
            while not all(o.is_ready() for o in outs):
                time.sleep(0.002)
        except Exception:
            pass
        res = [np.asarray(o) for o in outs]
        # recycle device output buffers as next call's donated out params
        # (every output element is written by the kernel each run)
        self._next_outbufs = list(outs)
        return dict(zip(self.out_names, res))


_CACHE = {}


def kernel(**inputs):
    import gc
    c = CFG_FULL
    if "exec" not in _CACHE:
        _CACHE["exec"] = _Exec(build_kernel(c))
    ex = _CACHE["exec"]
    gc_was_on = gc.isenabled()
    if gc_was_on:
        gc.disable()
    try:
        named = _prep(c, inputs, put=ex.put)
        res = ex(named)
    finally:
        if gc_was_on:
            gc.enable()
    out = res["out"].reshape(NC, c.DEC_NT * c.DEC_T)[:, :c.DEC_PC]
    return out.reshape(-1).astype(np.float32)


# revision 77
# speedup vs baseline: 1.0180x; 1.0180x over previous
"""GATv2 link-prediction network on 8 TRN2 NeuronCores.

Strategy (edge-parallel, dst-sharded):
  - Nodes padded to 50176 = 8 * 6272; core c owns dst range [c*6272, (c+1)*6272).
  - Edges (incl. self-loops) sorted by dst, assigned to the core owning dst,
    grouped into 49 dst-windows of 128 nodes, each padded to SB*128 edge slots.
  - Per layer: per-node tables xl = x@wl, xr = x@wr computed locally and
    AllGathered; per edge-subtile the src rows are fetched with streamed
    indirect DMAs; dst rows are expanded on-chip from the 128-row dst window
    with a selection-matrix matmul.
  - Attention logits: e = a . leaky_relu(u+v) via wide DVE ops; w = exp(e)
    (softmax max-subtraction dropped: |e| <= ~10 so fp32 exp is exact enough).
  - Segment softmax + aggregation fused into PSUM matmuls:
    psum[d, :] += (S_T * w).T @ [u | 1]  ->  z[d] = psum[:, :F]/psum[:, F] + b.
  - Decoder: z2 rows gathered per decode edge, MLP runs feature-major on PE.

Call-layer performance (the dominant cost: device exec is ~20ms, while the
loopback PJRT relay moves ~140MB/s with ~100ms blocking latency, all sharing
one host CPU): per-call bytes are minimized — x as fp8 e3m4 (exact-cast to
bf16 on-device), edge src ids as u16 + dst-local ids as u8 in one packed
byte tensor (bitcast + widened on-device), decode ids as u16, weights
uploaded once as a 16-row shard and AllGathered on-device, f32 constants in
one row (PE outer-product broadcast / transpose on-device), output as f16.
Host prep is fused into numba single-pass kernels and each tensor is
device_put asynchronously the moment it is assembled so upload streams
behind the remaining CPU work. The executor is built once and reused:
run_bass_kernel_spmd's axon path (bass2jax.run_bass_via_pjrt) re-creates the
jax.jit wrapper on every call, which costs seconds of re-trace/re-lower; we
inline that same path with a persistent jit. The donated output zero-buffers
are recycled from the previous call's device output (the kernel writes every
output element, so stale contents are harmless).
"""

import sys

sys.path.insert(0, "/opt/trn_rl_repo")

import numpy as np
import ml_dtypes

import concourse.bacc as bacc
import concourse.bass as bass
import concourse.mybir as mybir
import concourse.tile as tile

BF16 = mybir.dt.bfloat16
F32 = mybir.dt.float32
F16 = mybir.dt.float16
F8E3 = mybir.dt.float8e3
I32 = mybir.dt.int32
U16 = mybir.dt.uint16
U8 = mybir.dt.uint8

NC = 8
NEG_SLOPE = 0.2


class Cfg:
    def __init__(self, n=50000, e=1600000, e_dec=500000, in_c=128, hid=128,
                 out_c=64, sb=36, dec_t=512):
        self.N, self.E, self.E_DEC = n, e, e_dec
        self.IN_C, self.HID, self.OUT_C = in_c, hid, out_c
        self.NPC = ((n // NC + 127) // 128) * 128      # padded nodes per core
        self.G = self.NPC // 128                        # dst groups per core
        self.NP = self.NPC * NC                         # padded node count
        self.SB = sb                                    # subtiles per group
        self.W = sb * 128                               # edge slots per group
        self.DEC_T = dec_t                              # decode edges per tile
        dec_pc = (2 * e_dec) // NC
        self.DEC_PC = dec_pc
        self.DEC_NT = (dec_pc + dec_t - 1) // dec_t     # decode tiles per core
        self.DEC_PAD = self.DEC_NT * dec_t


CFG_FULL = Cfg()

try:
    import numba

    @numba.njit(cache=True)
    def _edge_pack(e0, e1, n_nodes, ED, DLR, npcr, NPC, SB, ngg):
        """Count-then-scatter edge grouping (incl. self-loops), one core pass.

        ED is the flat uint8 view of [ngg, 128, 2*SB]: little-endian u16 src
        offsets. DLR is the flat view of [ngg, SB*128]: j-major dst-local
        ids (255 = padding, pre-filled).
        """
        ne = e0.shape[0]
        cursor = np.zeros(ngg, np.int32)
        row = 2 * SB
        for e in range(ne):
            s = e0[e]
            d = e1[e]
            qs = s // npcr
            sp = qs * NPC + (s - qs * npcr)
            qd = d // npcr
            dp = qd * NPC + (d - qd * npcr)
            g = dp >> 7
            slot = cursor[g]
            cursor[g] = slot + 1
            base = (g * 128 + (slot & 127)) * row
            j = slot >> 7
            ED[base + 2 * j] = sp & 255
            ED[base + 2 * j + 1] = sp >> 8
            DLR[(g * SB + j) * 128 + (slot & 127)] = dp & 127
        for i in range(n_nodes):
            q = i // npcr
            dp = q * NPC + (i - q * npcr)
            g = dp >> 7
            slot = cursor[g]
            cursor[g] = slot + 1
            base = (g * 128 + (slot & 127)) * row
            j = slot >> 7
            ED[base + 2 * j] = dp & 255
            ED[base + 2 * j + 1] = dp >> 8
            DLR[(g * SB + j) * 128 + (slot & 127)] = dp & 127
        mx = 0
        for g in range(ngg):
            if cursor[g] > mx:
                mx = cursor[g]
        return mx
    @numba.njit(cache=True)
    def _x_to_f8_nb(xu, XL, lut, n_nodes, npcr, NPC, in_c):
        """f32 bits -> round-to-nearest bf16 index -> f8e3m4 byte, written
        into the node-padded per-core layout in one pass."""
        for n_ in range(n_nodes):
            cc = n_ // npcr
            ob = (cc * NPC + (n_ - cc * npcr)) * in_c
            ib = n_ * in_c
            for f in range(in_c):
                XL[ob + f] = lut[(xu[ib + f] + 0x8000) >> 16]

    @numba.njit(cache=True)
    def _dec_pack2_nb(pa, pb, na, nb, out_, npcr, NPC, DEC_PC, DEC_T,
                      DEC_NT, DGC):
        """Both sides of pos then neg decode edges in two passes; writing
        side a and b of the same edge back-to-back hits the same cache line
        of out_ ([tile, p, side, k] layout)."""
        ed = pa.shape[0]
        for half in range(2):
            ia = pa if half == 0 else na
            ib = pb if half == 0 else nb
            off = half * ed
            for i in range(ed):
                gi = off + i
                c = gi // DEC_PC
                ric = gi - c * DEC_PC
                t = ric // DEC_T
                r = ric - t * DEC_T
                base = ((c * DEC_NT + t) * 128 + (r & 127)) * 2 * DGC + (r >> 7)
                v = ia[i]
                q = v // npcr
                out_[base] = q * NPC + (v - q * npcr)
                v = ib[i]
                q = v // npcr
                out_[base + DGC] = q * NPC + (v - q * npcr)
except ImportError:  # pragma: no cover - numba always present in container
    _edge_pack = None
    _dec_pack2_nb = None
    _x_to_f8_nb = None


_F8LUT = None


def _f8lut():
    """u8 f8e3m4 bytes indexed by bf16 bit pattern."""
    global _F8LUT
    if _F8LUT is None:
        allbf = (np.arange(65536, dtype=np.uint32) << 16).view(np.float32)
        _F8LUT = np.ascontiguousarray(
            allbf.astype(ml_dtypes.float8_e3m4).view(np.uint8))
    return _F8LUT


def build_kernel(c: Cfg):
    nc = bacc.Bacc("TRN2", num_devices=NC)
    SB, G, NPC, NP = c.SB, c.G, c.NPC, c.NP
    IN_C, HID, OUT_C = c.IN_C, c.HID, c.OUT_C
    DEC_T, DEC_NT = c.DEC_T, c.DEC_NT
    DGC = DEC_T // 128                                  # gather calls per side per tile

    # ---- I/O (5 inputs: per-put issue overhead over the tunnel is ~15ms,
    # so everything small is packed into wpack/cvec) ----
    x_loc = nc.dram_tensor("x_loc", [NPC, IN_C], F8E3, kind="ExternalInput")
    # per group: 2*SB bytes of u16 src offsets per partition row
    edges = nc.dram_tensor("edges", [G, 128, 2 * SB], U8, kind="ExternalInput")
    # dst-local ids, j-major flat row per group (plain 2-D DMA loads only:
    # a 1-D rearranged-AP DMA makes the NEFF unloadable)
    dlrow = nc.dram_tensor("dlrow", [G, SB * 128], U8, kind="ExternalInput")
    # per decode tile: DGC cols of side-a offsets | DGC cols of side-b
    offs_ab = nc.dram_tensor("offs_ab", [DEC_NT, 128, 2 * DGC], U16,
                             kind="ExternalInput")
    # w1lr | w2lr | fw1 | fw2(pad) | fw3 | fw4(pad) — each core gets a
    # 16-row slice; the full [128, 704] is AllGathered on-device
    wpack = nc.dram_tensor("wpack", [128 // NC, 704], BF16,
                           kind="ExternalInput")
    wscr = nc.dram_tensor("wscr", [128 // NC, 704], BF16)
    wfull = nc.dram_tensor("wfull", [128, 704], BF16, addr_space="Shared")
    # a1|b1|a2|b2 | fb1(pad) | fb2 | fb3(pad) | fb4(pad)
    cvec = nc.dram_tensor("cvec", [1, 1024], F32, kind="ExternalInput")
    out = nc.dram_tensor("out", [DEC_NT, DEC_T], F16, kind="ExternalOutput")

    # internal DRAM
    xl1_loc = nc.dram_tensor("xl1_loc", [NPC, HID], BF16)
    xr1_loc = nc.dram_tensor("xr1_loc", [NPC, HID], BF16)
    xl1 = nc.dram_tensor("xl1", [NP, HID], BF16, addr_space="Shared")
    z1_loc = nc.dram_tensor("z1_loc", [NPC, HID], BF16)
    xl2_loc = nc.dram_tensor("xl2_loc", [NPC, OUT_C], BF16)
    xl2 = nc.dram_tensor("xl2", [NP, OUT_C], BF16, addr_space="Shared")
    xr2_loc = nc.dram_tensor("xr2_loc", [NPC, OUT_C], BF16)
    z2_loc = nc.dram_tensor("z2_loc", [NPC, OUT_C], BF16)
    z2 = nc.dram_tensor("z2", [NP, OUT_C], BF16, addr_space="Shared")

    rg = [list(range(NC))]

    with tile.TileContext(nc) as tc:
        with tc.tile_pool(name="const", bufs=1) as cp, \
             tc.tile_pool(name="sb", bufs=2) as sp, \
             tc.tile_pool(name="wide", bufs=2) as wp, \
             tc.tile_pool(name="wide1", bufs=1) as wq, \
             tc.tile_pool(name="ps", bufs=2, space="PSUM") as pp, \
             tc.tile_pool(name="ps2", bufs=2, space="PSUM") as pp2, \
             tc.tile_pool(name="ps3", bufs=3, space="PSUM") as pp3:

            ident = cp.tile([128, 128], BF16, tag="ident")
            from concourse.masks import make_identity
            make_identity(nc, ident[:])
            # iota row 0..127, same on every partition, generated on-device
            iota_i = cp.tile([128, 128], I32, tag="iota_i")
            nc.gpsimd.iota(iota_i[:], pattern=[[1, 128]], base=0,
                           channel_multiplier=0)
            iota_t = cp.tile([128, 128], BF16, tag="iota")
            nc.vector.tensor_copy(out=iota_t[:], in_=iota_i[:])
            iota_wi = sp.tile([128, SB * 128], I32, tag="iota_wi")
            nc.gpsimd.iota(iota_wi[:], pattern=[[0, SB * 128]], base=0,
                           channel_multiplier=1)
            iota_w = cp.tile([128, SB * 128], BF16, tag="iota_w")
            nc.vector.tensor_copy(out=iota_w[:], in_=iota_wi[:])
            # one upload slice per core -> AllGather -> one DMA for all weights
            # (collectives can't read IO tensors: stage via SBUF -> scratch)
            wsl = cp.tile([128 // NC, 704], BF16, tag="wslice")
            nc.sync.dma_start(out=wsl[:], in_=wpack[:])
            nc.sync.dma_start(out=wscr[:], in_=wsl[:])
            nc.gpsimd.collective_compute(
                "AllGather", mybir.AluOpType.bypass,
                replica_groups=[list(range(NC))],
                ins=[wscr[:]], outs=[wfull[:]])
            wl_t = cp.tile([128, 704], BF16, tag="wpack")
            nc.sync.dma_start(out=wl_t[:], in_=wfull[:])
            w1_t = wl_t[:, 0:256]
            w2_t = wl_t[:, 256:384]
            fw1_t = wl_t[:, 384:448]
            fw2_t = wl_t[:64, 448:576]
            fw3_t = wl_t[:, 576:640]
            fw4_t = wl_t[:64, 640:704]
            cv_t = cp.tile([1, 1024], F32, tag="cvec")
            nc.sync.dma_start(out=cv_t[:], in_=cvec[:])
            # broadcast a1/b1/a2/b2 rows [1,128] -> [128,128] via PE outer product
            ones1 = cp.tile([1, 128], F32, tag="ones1")
            nc.vector.memset(ones1[:], 1.0)
            ab_bc = []
            for i in range(4):
                psb = pp3.tile([128, 128], F32, tag="C")
                nc.tensor.matmul(out=psb[:], lhsT=ones1[:],
                                 rhs=cv_t[0:1, i * 128:(i + 1) * 128],
                                 start=True, stop=True)
                tbc = cp.tile([128, 128], F32, tag=f"abbc{i}")
                nc.vector.tensor_copy(out=tbc[:], in_=psb[:])
                ab_bc.append(tbc)
            a1_t, b1_t, a2_t, b2_t = ab_bc
            # decoder bias columns: PE-transpose each [1,128] row to [128,1]
            ones11 = cp.tile([1, 1], F32, tag="ones11")
            nc.vector.memset(ones11[:], 1.0)
            fb_t = cp.tile([128, 4], F32, tag="fb")
            for i in range(4):
                psc = pp3.tile([128, 128], F32, tag="C")
                nc.tensor.transpose(out=psc[:, 0:1],
                                    in_=cv_t[0:1, 512 + i * 128:
                                             512 + (i + 1) * 128],
                                    identity=ones11[:])
                nc.vector.tensor_copy(out=fb_t[:, i:i + 1], in_=psc[:, 0:1])

            def tables(src_dram, w_t, fin, fout2, dst_l, dst_r, in_dt=BF16):
                """dst_l[i] | dst_r[i] = (src[i*128:...]) @ [wl | wr]."""
                ntile = src_dram.shape[0] // 128
                for i in range(ntile):
                    if in_dt is BF16:
                        xt = sp.tile([128, fin], BF16, tag="tab_x")
                        nc.sync.dma_start(out=xt[:],
                                          in_=src_dram[i * 128:(i + 1) * 128, :])
                    else:
                        x8 = sp.tile([128, fin], in_dt, tag="tab_x8")
                        nc.sync.dma_start(out=x8[:],
                                          in_=src_dram[i * 128:(i + 1) * 128, :])
                        xt = sp.tile([128, fin], BF16, tag="tab_x")
                        nc.vector.tensor_copy(out=xt[:], in_=x8[:])
                    xtt = pp.tile([fin, 128], BF16, tag="A")
                    nc.tensor.transpose(out=xtt[:], in_=xt[:], identity=ident[:])
                    xts = sp.tile([fin, 128], BF16, tag="tab_Ts")
                    nc.vector.tensor_copy(out=xts[:], in_=xtt[:])
                    op = pp2.tile([128, fout2], F32, tag="B")
                    nc.tensor.matmul(out=op[:], lhsT=xts[:], rhs=w_t[:],
                                     start=True, stop=True)
                    os_ = sp.tile([128, fout2], BF16, tag="tab_os")
                    nc.vector.tensor_copy(out=os_[:], in_=op[:])
                    nc.sync.dma_start(out=dst_l[i * 128:(i + 1) * 128, :],
                                      in_=os_[:, :fout2 // 2])
                    nc.sync.dma_start(out=dst_r[i * 128:(i + 1) * 128, :],
                                      in_=os_[:, fout2 // 2:])

            def allgather(loc, full):
                nc.gpsimd.collective_compute(
                    "AllGather", mybir.AluOpType.bypass, replica_groups=rg,
                    ins=[loc[:]], outs=[full[:]])

            def edge_layer(ul_tab, vloc_tab, F_, a_t, b_t, relu, z_out):
                """One GATv2 layer edge pass. F_ = feature width."""
                FE = F_ + 4                      # u tile row: F_ feats + 1.0 col + pad
                for g in range(G):
                    ed = sp.tile([128, 2 * SB], U8, tag="edges8")
                    nc.gpsimd.dma_start(out=ed[:], in_=edges[g])
                    ou = sp.tile([128, SB], I32, tag="offu")
                    nc.vector.tensor_copy(out=ou[:], in_=ed[:, :].bitcast(U16))
                    dlf8 = sp.tile([1, SB * 128], U8, tag="dlf8")
                    nc.sync.dma_start(out=dlf8[:], in_=dlrow[g:g + 1, :])
                    dlf = sp.tile([1, SB * 128], BF16, tag="dlf")
                    nc.vector.tensor_copy(out=dlf[:], in_=dlf8[:])
                    dlbc = wq.tile([128, SB * 128], BF16, tag="dlbc")
                    nc.gpsimd.partition_broadcast(dlbc[:], dlf[:])
                    dlj8 = sp.tile([SB, 128], U8, tag="dlj8")
                    nc.sync.dma_start(
                        out=dlj8[:],
                        in_=dlrow[g:g + 1, :].rearrange("o (j s) -> (o j) s",
                                                        j=SB))
                    dljb = sp.tile([SB, 128], BF16, tag="dljb")
                    nc.vector.tensor_copy(out=dljb[:], in_=dlj8[:])
                    dlp = pp3.tile([128, 128], BF16, tag="C")
                    nc.tensor.transpose(out=dlp[:, :SB], in_=dljb[:],
                                        identity=ident[:SB, :SB])
                    dl = sp.tile([128, SB], BF16, tag="dstloc")
                    nc.vector.tensor_copy(out=dl[:], in_=dlp[:, :SB])
                    u = wp.tile([128, SB * FE], BF16, tag="u")
                    u3 = u[:].rearrange("p (j f) -> p j f", j=SB)
                    nc.vector.memset(u3[:, :, F_:F_ + 1], 1.0)
                    for j in range(SB):
                        nc.gpsimd.indirect_dma_start(
                            out=u3[:, j, :F_], out_offset=None, in_=ul_tab[:],
                            in_offset=bass.IndirectOffsetOnAxis(
                                ap=ou[:, j:j + 1], axis=0))
                    t = wp.tile([128, SB * F_], F32, tag="t")
                    t3 = t[:].rearrange("p (j f) -> p j f", j=SB)
                    st = wp.tile([128, SB * 128], BF16, tag="st")
                    st3 = st[:].rearrange("p (j d) -> p j d", j=SB)
                    nc.vector.tensor_tensor(
                        out=st3[:, :, :],
                        in0=dl[:].rearrange("p (j o) -> p j o", o=1).to_broadcast([128, SB, 128]),
                        in1=iota_t[:].rearrange("p (o d) -> p o d", o=1).to_broadcast([128, SB, 128]),
                        op=mybir.AluOpType.is_equal)
                    # v rows for this dst window, expanded per-edge on PE.
                    # The selection matrix is built directly in transposed,
                    # j-major form (one wide compare vs 36 transposes+copies)
                    vg = sp.tile([128, F_], BF16, tag="vg")
                    nc.sync.dma_start(
                        out=vg[:], in_=vloc_tab[g * 128:(g + 1) * 128, :])
                    stT = wq.tile([128, SB * 128], BF16, tag="stT")
                    stT3 = stT[:].rearrange("p (j s) -> p j s", j=SB)
                    nc.vector.tensor_tensor(out=stT[:], in0=dlbc[:],
                                            in1=iota_w[:],
                                            op=mybir.AluOpType.is_equal)
                    for j in range(SB):
                        vp = pp2.tile([128, F_], F32, tag="B")
                        nc.tensor.matmul(out=vp[:], lhsT=stT3[:, j, :],
                                         rhs=vg[:], start=True, stop=True)
                        nc.vector.tensor_add(out=t3[:, j, :],
                                             in0=u3[:, j, :F_], in1=vp[:])
                    nc.vector.scalar_tensor_tensor(
                        out=t[:], in0=t[:], scalar=float(NEG_SLOPE), in1=t[:],
                        op0=mybir.AluOpType.mult, op1=mybir.AluOpType.max)
                    # t is dead after the a-weighting: multiply in place
                    ta = t
                    nc.vector.tensor_tensor(
                        out=ta[:].rearrange("p (j f) -> p j f", j=SB),
                        in0=t3[:, :, :],
                        in1=a_t[:, :F_].rearrange("p (o f) -> p o f", o=1).to_broadcast([128, SB, F_]),
                        op=mybir.AluOpType.mult)
                    ev = sp.tile([128, SB], F32, tag="ev")
                    nc.vector.tensor_reduce(
                        out=ev[:], in_=ta[:].rearrange("p (j f) -> p j f", j=SB),
                        axis=mybir.AxisListType.X, op=mybir.AluOpType.add)
                    wv = sp.tile([128, SB], F32, tag="wv")
                    nc.scalar.activation(wv[:], ev[:],
                                         mybir.ActivationFunctionType.Exp)
                    # S' = S_T * w  (broadcast w along d)
                    nc.vector.tensor_tensor(
                        out=st3[:, :, :], in0=st3[:, :, :],
                        in1=wv[:].rearrange("p (j o) -> p j o", o=1).to_broadcast([128, SB, 128]),
                        op=mybir.AluOpType.mult)
                    acc = pp.tile([128, F_ + 4], F32, tag="A")
                    for j in range(SB):
                        nc.tensor.matmul(
                            out=acc[:, :F_ + 1], lhsT=st3[:, j, :],
                            rhs=u3[:, j, :F_ + 1],
                            start=(j == 0), stop=(j == SB - 1))
                    den = sp.tile([128, 1], F32, tag="den")
                    nc.vector.tensor_scalar_add(den[:], acc[:, F_:F_ + 1], 1e-30)
                    rec = sp.tile([128, 1], F32, tag="rec")
                    nc.vector.reciprocal(rec[:], den[:])
                    zt = sp.tile([128, F_], F32, tag="zt")
                    nc.vector.scalar_tensor_tensor(
                        out=zt[:], in0=acc[:, :F_], scalar=rec[:, :1], in1=b_t[:, :F_],
                        op0=mybir.AluOpType.mult, op1=mybir.AluOpType.add)
                    zb = sp.tile([128, F_], BF16, tag="zb")
                    if relu:
                        nc.scalar.activation(zb[:], zt[:],
                                             mybir.ActivationFunctionType.Relu)
                    else:
                        nc.vector.tensor_copy(out=zb[:], in_=zt[:])
                    nc.sync.dma_start(out=z_out[g * 128:(g + 1) * 128, :], in_=zb[:])

            # ---- phase A: L1 tables (xr1 only needed locally) ----
            tables(x_loc, w1_t, IN_C, 2 * HID, xl1_loc, xr1_loc, in_dt=F8E3)
            allgather(xl1_loc, xl1)
            # ---- phase B: L1 edges ----
            edge_layer(xl1, xr1_loc, HID, a1_t, b1_t, True, z1_loc)
            # ---- phase D: L2 tables, computed on local z1 rows only and
            # AllGathered (vs recomputing the full table on every core) ----
            tables(z1_loc, w2_t, HID, 2 * OUT_C, xl2_loc, xr2_loc)
            allgather(xl2_loc, xl2)
            # ---- phase E: L2 edges ----
            edge_layer(xl2, xr2_loc, OUT_C, a2_t, b2_t, False, z2_loc)
            allgather(z2_loc, z2)

            # ---- decoder ----
            for tdx in range(DEC_NT):
                ab16 = sp.tile([128, 2 * DGC], U16, tag="offab16")
                nc.gpsimd.dma_start(out=ab16[:], in_=offs_ab[tdx])
                oa = sp.tile([128, DGC], I32, tag="offa")
                nc.vector.tensor_copy(out=oa[:], in_=ab16[:, :DGC])
                ob = sp.tile([128, DGC], I32, tag="offb")
                nc.vector.tensor_copy(out=ob[:], in_=ab16[:, DGC:])
                h = wp.tile([128, DGC * 2 * OUT_C], BF16, tag="h")
                h3 = h[:].rearrange("p (k f) -> p k f", k=DGC)
                for k in range(DGC):
                    nc.gpsimd.indirect_dma_start(
                        out=h3[:, k, :OUT_C], out_offset=None, in_=z2[:],
                        in_offset=bass.IndirectOffsetOnAxis(ap=oa[:, k:k + 1], axis=0))
                    nc.gpsimd.indirect_dma_start(
                        out=h3[:, k, OUT_C:], out_offset=None, in_=z2[:],
                        in_offset=bass.IndirectOffsetOnAxis(ap=ob[:, k:k + 1], axis=0))
                hT = sp.tile([128, DEC_T], BF16, tag="hT")
                for k in range(DGC):
                    htp = pp3.tile([128, 128], BF16, tag="C")
                    nc.tensor.transpose(out=htp[:], in_=h3[:, k, :], identity=ident[:])
                    nc.vector.tensor_copy(out=hT[:, k * 128:(k + 1) * 128], in_=htp[:])
                p1 = pp.tile([OUT_C, DEC_T], F32, tag="A")
                nc.tensor.matmul(out=p1[:], lhsT=fw1_t[:], rhs=hT[:], start=True, stop=True)
                s1 = sp.tile([OUT_C, DEC_T], BF16, tag="mlps1")
                nc.scalar.activation(s1[:], p1[:], mybir.ActivationFunctionType.Relu,
                                     bias=fb_t[:OUT_C, 0:1])
                p2 = pp2.tile([128, DEC_T], F32, tag="B")
                nc.tensor.matmul(out=p2[:], lhsT=fw2_t[:], rhs=s1[:], start=True, stop=True)
                s2 = sp.tile([128, DEC_T], BF16, tag="mlps2")
                nc.scalar.activation(s2[:], p2[:], mybir.ActivationFunctionType.Relu,
                                     bias=fb_t[:128, 1:2])
                p3 = pp3.tile([64, DEC_T], F32, tag="C")
                nc.tensor.matmul(out=p3[:], lhsT=fw3_t[:], rhs=s2[:], start=True, stop=True)
                s3 = sp.tile([64, DEC_T], BF16, tag="mlps3")
                nc.scalar.activation(s3[:], p3[:], mybir.ActivationFunctionType.Relu,
                                     bias=fb_t[:64, 2:3])
                p4 = pp.tile([64, DEC_T], F32, tag="A")
                nc.tensor.matmul(out=p4[:], lhsT=fw4_t[:], rhs=s3[:], start=True, stop=True)
                s4 = sp.tile([1, DEC_T], F16, tag="s4")
                nc.vector.tensor_scalar_add(s4[:], p4[:1, :], fb_t[:1, 3:4])
                nc.sync.dma_start(out=out[tdx:tdx + 1, :], in_=s4[:])

    nc.compile()
    return nc


# ---------------- host side ----------------

def _prep(c: Cfg, inputs, put=lambda a: a):
    """Shard + pad inputs; returns {name: array}.

    `put` is applied to each finished tensor immediately, so an async
    jax.device_put can stream earlier tensors while later ones are still
    being assembled on the CPU (x_loc is 37% of the bytes and is ready
    first; the edge grouping below then overlaps its upload).
    """
    bf = ml_dtypes.bfloat16
    N, NPC, G, SB, NP = c.N, c.NPC, c.G, c.SB, c.NP
    npc_real = N // NC
    named = {}

    def pid(n):
        q, r = np.divmod(n.astype(np.int32, copy=False), npc_real)
        return q * NPC + r

    # ---- replicated weights (near-instant: gets the tunnel streaming) ----
    def rep(a):
        return np.ascontiguousarray(np.broadcast_to(a, (NC,) + a.shape)).reshape(
            (NC * a.shape[0],) + a.shape[1:])

    wp_ = np.zeros((128, 704), np.float32)
    wp_[:, 0:128] = np.asarray(inputs["w1l"], np.float32)
    wp_[:, 128:256] = np.asarray(inputs["w1r"], np.float32)
    wp_[:, 256:320] = np.asarray(inputs["w2l"], np.float32)
    wp_[:, 320:384] = np.asarray(inputs["w2r"], np.float32)
    wp_[:, 384:448] = np.asarray(inputs["fw1"], np.float32)
    wp_[:64, 448:576] = np.asarray(inputs["fw2"], np.float32)
    wp_[:, 576:640] = np.asarray(inputs["fw3"], np.float32)
    wp_[:64, 640:641] = np.asarray(inputs["fw4"], np.float32)
    named["wpack"] = put(wp_.astype(bf))  # sharded 16 rows/core, allgathered
    cv = np.zeros((1, 1024), np.float32)
    cv[0, 0:c.HID] = np.asarray(inputs["a1"], np.float32)
    cv[0, 128:128 + c.HID] = np.asarray(inputs["b1"], np.float32)
    cv[0, 256:256 + c.OUT_C] = np.asarray(inputs["a2"], np.float32)
    cv[0, 384:384 + c.OUT_C] = np.asarray(inputs["b2"], np.float32)
    cv[0, 512:512 + c.OUT_C] = np.asarray(inputs["fb1"], np.float32)
    cv[0, 640:768] = np.asarray(inputs["fb2"], np.float32)
    cv[0, 768:768 + 64] = np.asarray(inputs["fb3"], np.float32)
    cv[0, 896:897] = np.asarray(inputs["fb4"], np.float32)
    named["cvec"] = put(rep(cv))

    # ---- nodes (cheap to build, big to ship) ----
    x = np.ascontiguousarray(np.asarray(inputs["x"], np.float32))
    lut = _f8lut()
    XL = np.zeros(NC * NPC * c.IN_C, np.uint8)
    if _x_to_f8_nb is not None:
        _x_to_f8_nb(x.reshape(-1).view(np.uint32), XL, lut, N, npc_real,
                    NPC, c.IN_C)
    else:
        idx = (x.reshape(-1).view(np.uint32) + 0x8000) >> 16
        XLv = XL.reshape(NC, NPC, c.IN_C)
        XLv[:, :npc_real] = lut[idx].reshape(NC, npc_real, c.IN_C)
    named["x_loc"] = put(
        XL.view(ml_dtypes.float8_e3m4).reshape(NC * NPC, c.IN_C))

    # ---- decode edges, data-parallel ----
    pe = np.asarray(inputs["pos_edge_index"])
    ne = np.asarray(inputs["neg_edge_index"])
    DGC = c.DEC_T // 128
    OAB = np.zeros(NC * c.DEC_NT * 128 * 2 * DGC, np.uint16)
    if _dec_pack2_nb is not None:
        _dec_pack2_nb(np.ascontiguousarray(pe[0].astype(np.int32, copy=False)),
                      np.ascontiguousarray(pe[1].astype(np.int32, copy=False)),
                      np.ascontiguousarray(ne[0].astype(np.int32, copy=False)),
                      np.ascontiguousarray(ne[1].astype(np.int32, copy=False)),
                      OAB, npc_real, NPC, c.DEC_PC, c.DEC_T, c.DEC_NT, DGC)
    else:
        v = OAB.reshape(NC * c.DEC_NT, 2, DGC, 128)  # [tile, side, k, p]
        for side, (p_, n_) in enumerate([(pe[0], ne[0]), (pe[1], ne[1])]):
            ids = np.concatenate([p_, n_]).astype(np.int32, copy=False)
            arr = np.zeros((NC, c.DEC_PAD), np.uint16)
            arr[:, :c.DEC_PC] = pid(ids).reshape(NC, c.DEC_PC)
            v[:, side] = arr.reshape(NC * c.DEC_NT, DGC, 128)
        OAB = np.ascontiguousarray(
            v.transpose(0, 3, 1, 2)).reshape(-1)  # -> [tile, p, side, k]
    named["offs_ab"] = put(OAB.reshape(NC * c.DEC_NT, 128, 2 * DGC))

    # ---- message edges, grouped by 128-wide dst window ----
    ei = np.asarray(inputs["edge_index"])
    e0 = np.ascontiguousarray(ei[0].astype(np.int32, copy=False))
    e1 = np.ascontiguousarray(ei[1].astype(np.int32, copy=False))
    ngg = NC * G
    ED = np.zeros(ngg * 128 * 2 * SB, np.uint8)
    DLR = np.full(ngg * SB * 128, 255, np.uint8)
    if _edge_pack is not None:
        mx = _edge_pack(e0, e1, N, ED, DLR, npc_real, NPC, SB, ngg)
        assert mx <= SB * 128, f"group overflow: {mx} > {SB * 128}"
    else:
        loops = np.arange(N, dtype=np.int32)
        src = np.concatenate([e0, loops])
        dst = np.concatenate([e1, loops])
        sp_, dp = pid(src), pid(dst)
        gg_e = (dp >> 7).astype(np.uint16)   # NPC % 128 == 0 -> global group id
        order = np.argsort(gg_e, kind="stable")
        sp_s = sp_[order].astype(np.uint16)
        dl_s = (dp[order] & 127).astype(np.uint8)
        counts = np.bincount(gg_e, minlength=ngg)
        assert counts.max() <= SB * 128, \
            f"group overflow: {counts.max()} > {SB * 128}"
        starts = np.zeros(ngg, np.int32)
        np.cumsum(counts[:-1], out=starts[1:], dtype=np.int32)
        slot = np.arange(dp.shape[0], dtype=np.int32) - np.repeat(starts, counts)
        gg = gg_e[order].astype(np.int32)
        rowbase = (gg * 128 + slot % 128) * (2 * SB)
        j_ = slot // 128
        ED[rowbase + 2 * j_] = (sp_s & 255).astype(np.uint8)
        ED[rowbase + 2 * j_ + 1] = (sp_s >> 8).astype(np.uint8)
        DLR[(gg * SB + j_) * 128 + slot % 128] = dl_s
    named["edges"] = put(ED.reshape(ngg, 128, 2 * SB))
    named["dlrow"] = put(DLR.reshape(ngg, SB * 128))
    return named


class _Exec:
    """Persistent jit wrapper around the bass NEFF (the same PJRT path
    run_bass_kernel_spmd takes under axon, minus the per-call re-trace)."""

    def __init__(self, nc):
        import jax
        from jax.sharding import Mesh, PartitionSpec
        from jax.experimental.shard_map import shard_map
        from concourse import bass2jax

        bass2jax.install_neuronx_cc_hook()
        self.jax = jax
        partition_name = (nc.partition_id_tensor.name
                          if nc.partition_id_tensor else None)
        in_names, out_names, out_avals, zero_outs = [], [], [], []
        for alloc in nc.m.functions[0].allocations:
            if not isinstance(alloc, mybir.MemoryLocationSet):
                continue
            name = alloc.memorylocations[0].name
            if alloc.kind == "ExternalInput":
                if name != partition_name:
                    in_names.append(name)
            elif alloc.kind == "ExternalOutput":
                shape = tuple(alloc.tensor_shape)
                dtype = mybir.dt.np(alloc.dtype)
                out_names.append(name)
                out_avals.append(jax.core.ShapedArray(shape, dtype))
                zero_outs.append(
                    np.zeros((NC * shape[0], *shape[1:]), dtype))
        n_params = len(in_names)
        self.in_names = list(in_names)
        self.out_names = out_names
        all_names = in_names + out_names
        if partition_name is not None:
            all_names.append(partition_name)
        donate = tuple(range(n_params, n_params + len(out_names)))

        def _body(*args):
            operands = list(args)
            if partition_name is not None:
                operands.append(bass2jax.partition_id_tensor())
            return tuple(_bind(*operands))

        def _bind(*operands):
            return bass2jax._bass_exec_p.bind(
                *operands, out_avals=tuple(out_avals),
                in_names=tuple(all_names), out_names=tuple(out_names),
                lowering_input_output_aliases=(), sim_require_finite=True,
                sim_require_nnan=True, nc=nc)

        devices = jax.devices()[:NC]
        mesh = Mesh(np.asarray(devices), ("core",))
        specs = (PartitionSpec("core"),)
        self.sharded = jax.jit(
            shard_map(_body, mesh=mesh,
                      in_specs=specs * (n_params + len(out_names)),
                      out_specs=specs * len(out_names), check_rep=False),
            donate_argnums=donate, keep_unused=True)
        # pre-place the first call's donated out-buffers so every call sees
        # device-array outbufs (one jit signature, no second XLA compile)
        from jax.sharding import NamedSharding
        self.shd = NamedSharding(mesh, PartitionSpec("core"))
        self._next_outbufs = [jax.device_put(z, self.shd) for z in zero_outs]

    def put(self, arr):
        return self.jax.device_put(arr, self.shd)

    def __call__(self, named):
        import time
        args = [named[n] for n in self.in_names]
        outs = self.sharded(*args, *self._next_outbufs)
        # prefetch D2H, then wait by yielding: the blocking asarray path
        # busy-holds the only CPU that the loopback relay needs to finish
        # streaming; a sleep-spin is ~10-15ms faster end-to-end
        try:
            for o in outs:
                o.copy_to_host_async()
            while not all(o.is_ready() for o in outs):
                time.sleep(0.001)
        except Exception:
            pass
        res = [np.asarray(o) for o in outs]
        # recycle device output buffers as next call's donated out params
        # (every output element is written by the kernel each run)
        self._next_outbufs = list(outs)
        return dict(zip(self.out_names, res))


_CACHE = {}


def kernel(**inputs):
    import gc
    c = CFG_FULL
    if "exec" not in _CACHE:
        _CACHE["exec"] = _Exec(build_kernel(c))
    ex = _CACHE["exec"]
    gc_was_on = gc.isenabled()
    if gc_was_on:
        gc.disable()
    try:
        named = _prep(c, inputs, put=ex.put)
        res = ex(named)
    finally:
        if gc_was_on:
            gc.enable()
    # slice + cast in one pass (astype of the strided view), then flatten
    out = res["out"].reshape(NC, c.DEC_NT * c.DEC_T)[:, :c.DEC_PC]
    return out.astype(np.float32).reshape(-1)
